# revision 13
# baseline (speedup 1.0000x reference)
"""Trainium2 Bass kernel for the AGRAN anchor attention module.

Reference math (per batch b, heads h=2, d=64, L=200, H=128):
  Q = (queries @ Wq.T + bq) * 1/sqrt(d)          [b, l, h*d]
  K' = keys @ Wk.T + bk + abs_pos_K              [b, l, h*d]
  V' = keys @ Wv.T + bv + abs_pos_V              [b, l, h*d]
  S[h,q,k] = Q_h[q]·K'_h[k] + sum_d (tmK+dmK)[q,k,h,d]·Q_h[q,d]
  S masked by (time_mask row | causal), softmax over k -> A
  out[h,q] = sum_k A[h,q,k] * (V'_h[k] + (tmV+dmV)[q,k,h,:])

Sharding: data-parallel over batch B=16 across 8 cores (2 per core),
weights/masks replicated.  No collectives; the host concatenates shards.

Per-core dataflow:
  * K-side relation tensors stream as [q-partition, (k,hd)-free] chunks
    through the vector engine (add, broadcast-mult by scaled Q, segmented
    reduce over d).
  * V-side relation tensors load as per-(b,q) [k-partition, tm|dm] tiles;
    PE contracts over k with 2-column A slices as stationary weights
    (float32r, single pass).  The [2, 256] PSUM strips are scattered by
    GPSIMD DMAs into [q, hd] tiles.
  * DMA issue is spread over the Sync/Act HWDGE rings and 8 SWDGE queues.
"""

import numpy as np

B, L, H, NH, D = 16, 200, 128, 2, 64
NCORES = 8
B_SH = B // NCORES  # 2 batches per core
NEGC = -30000.0     # masked-score fill (any row-constant that underflows exp)
SCALE = 0.125       # 1/sqrt(D)
PT = [(0, 128), (128, 72)]  # partition tiling of the 200-long seq dim
KC = 25             # k-chunk for the K-side relation streaming
NKC = L // KC       # 8 chunks
QG = 4              # q's per PSUM strip tile on the V side

_CACHE = {}


def _build_nc():
    import concourse.bass as bass
    import concourse.bacc as bacc
    import concourse.tile as tile
    from concourse import mybir
    from concourse.masks import make_identity
    from contextlib import ExitStack

    f32 = mybir.dt.float32
    f32r = mybir.dt.float32r
    bf16 = mybir.dt.bfloat16
    u8 = mybir.dt.uint8
    AF = mybir.ActivationFunctionType
    OP = mybir.AluOpType
    AX = mybir.AxisListType

    nc = bacc.Bacc(num_swdge_queues=4)

    d_q = nc.declare_dram_parameter("queries", [B_SH, L, H], f32, isOutput=False)
    d_k = nc.declare_dram_parameter("keys", [B_SH, L, H], f32, isOutput=False)
    d_tmK = nc.declare_dram_parameter("time_matrix_K", [B_SH, L, L, H], f32, isOutput=False)
    d_tmV = nc.declare_dram_parameter("time_matrix_V", [B_SH, L, L, H], f32, isOutput=False)
    d_dmK = nc.declare_dram_parameter("dis_matrix_K", [B_SH, L, L, H], f32, isOutput=False)
    d_dmV = nc.declare_dram_parameter("dis_matrix_V", [B_SH, L, L, H], f32, isOutput=False)
    d_pK = nc.declare_dram_parameter("abs_pos_K", [B_SH, L, H], f32, isOutput=False)
    d_pV = nc.declare_dram_parameter("abs_pos_V", [B_SH, L, H], f32, isOutput=False)
    d_wq = nc.declare_dram_parameter("Wq", [H, H], f32, isOutput=False)
    d_bq = nc.declare_dram_parameter("bq", [H], f32, isOutput=False)
    d_wk = nc.declare_dram_parameter("Wk", [H, H], f32, isOutput=False)
    d_bk = nc.declare_dram_parameter("bk", [H], f32, isOutput=False)
    d_wv = nc.declare_dram_parameter("Wv", [H, H], f32, isOutput=False)
    d_bv = nc.declare_dram_parameter("bv", [H], f32, isOutput=False)
    d_tm = nc.declare_dram_parameter("time_mask", [B_SH, L], u8, isOutput=False)
    d_am = nc.declare_dram_parameter("attn_mask", [L, L], u8, isOutput=False)
    d_out = nc.declare_dram_parameter("out", [B_SH, L, H], f32, isOutput=True)

    def ap_of(t, offset, pattern):
        return bass.AP(tensor=t.tensor if isinstance(t, bass.AP) else t, offset=offset, ap=pattern)

    def bc_mid(ap2d, count):
        """[P, F] -> [P, count(bcast), F] by inserting a step-0 free dim."""
        return bass.AP(tensor=ap2d.tensor, offset=ap2d.offset,
                       ap=[ap2d.ap[0], [0, count], ap2d.ap[1]])

    with tile.TileContext(nc) as tc, ExitStack() as ctx:
        consts = ctx.enter_context(tc.tile_pool(name="consts", bufs=1))
        bpool = ctx.enter_context(tc.tile_pool(name="bpool", bufs=2))
        kpool = ctx.enter_context(tc.tile_pool(name="kpool", bufs=2))
        vpool = ctx.enter_context(tc.tile_pool(name="vpool", bufs=6))
        small = ctx.enter_context(tc.tile_pool(name="small", bufs=3))
        ps_g = ctx.enter_context(tc.tile_pool(name="ps_g", bufs=2, space="PSUM"))
        ps_avq = ctx.enter_context(tc.tile_pool(name="ps_avq", bufs=2, space="PSUM"))
        ps_rp = ctx.enter_context(tc.tile_pool(name="ps_rp", bufs=2, space="PSUM"))

        # ---- constants ----
        ident = consts.tile([128, 128], f32, name="ident", tag="ident")
        make_identity(nc, ident)

        wt_tiles = []
        for nm, dram in (("wq", d_wq), ("wk", d_wk), ("wv", d_wv)):
            wnat = consts.tile([128, 128], f32, name=f"{nm}nat", tag=f"{nm}nat")
            nc.gpsimd.dma_start(out=wnat, in_=dram[:, :])
            wps = ps_g.tile([128, 128], f32, name=f"{nm}ps", tag="g")
            nc.tensor.transpose(wps, wnat, ident)
            wt = consts.tile([128, 128], f32, name=f"{nm}t", tag=f"{nm}t")
            nc.scalar.copy(out=wt, in_=wps)
            wt_tiles.append(wt)
        wqt, wkt, wvt = wt_tiles

        # bq broadcast to all partitions, pre-scaled by 1/8 (free-dim layout)
        bq8r = consts.tile([128, 128], f32, name="bq8r", tag="bq8r")
        nc.gpsimd.dma_start(out=bq8r, in_=ap_of(d_bq, 0, [[0, 128], [1, 128]]))
        nc.scalar.mul(out=bq8r, in_=bq8r, mul=SCALE)
        # bv broadcast (free-dim layout)
        bvr = consts.tile([128, 128], f32, name="bvr", tag="bvr")
        nc.gpsimd.dma_start(out=bvr, in_=ap_of(d_bv, 0, [[0, 128], [1, 128]]))
        # per-partition column biases ([H,1])
        bqc = consts.tile([128, 1], f32, name="bqc", tag="bqc")
        nc.gpsimd.dma_start(out=bqc, in_=ap_of(d_bq, 0, [[1, 128], [0, 1]]))
        bkc = consts.tile([128, 1], f32, name="bkc", tag="bkc")
        nc.gpsimd.dma_start(out=bkc, in_=ap_of(d_bk, 0, [[1, 128], [0, 1]]))

        # (1 - attn_mask) per q-tile (shared across b)
        omas = []
        for ti, (qs, qn) in enumerate(PT):
            amu = consts.tile([128, L], u8, name=f"amu{ti}", tag=f"amu{ti}")
            nc.gpsimd.dma_start(out=amu[0:qn, :], in_=d_am[qs:qs + qn, :])
            oma = consts.tile([128, L], f32, name=f"oma{ti}", tag=f"oma{ti}")
            nc.vector.tensor_copy(out=oma[0:qn, :], in_=amu[0:qn, :])
            nc.vector.tensor_scalar(out=oma[0:qn, :], in0=oma[0:qn, :],
                                    scalar1=-1.0, scalar2=1.0,
                                    op0=OP.mult, op1=OP.add)
            omas.append(oma)

        STATE = []
        for b in range(B_SH):
            # ---- load activations [l, hd] per seq tile ----
            q_sb, k_sb, pk_sb, pv_sb = [], [], [], []
            for ti, (qs, qn) in enumerate(PT):
                tq = bpool.tile([128, H], f32, name=f"tq{ti}", tag=f"tq{ti}")
                nc.gpsimd.dma_start(out=tq[0:qn, :], in_=d_q[b, qs:qs + qn, :])
                tk = bpool.tile([128, H], f32, name=f"tk{ti}", tag=f"tk{ti}")
                nc.gpsimd.dma_start(out=tk[0:qn, :], in_=d_k[b, qs:qs + qn, :])
                tpk = bpool.tile([128, H], f32, name=f"tpk{ti}", tag=f"tpk{ti}")
                nc.gpsimd.dma_start(out=tpk[0:qn, :], in_=d_pK[b, qs:qs + qn, :])
                tpv = bpool.tile([128, H], f32, name=f"tpv{ti}", tag=f"tpv{ti}")
                nc.gpsimd.dma_start(out=tpv[0:qn, :], in_=d_pV[b, qs:qs + qn, :])
                q_sb.append(tq); k_sb.append(tk); pk_sb.append(tpk); pv_sb.append(tpv)

            # ---- transpose activations to [hd, l] ----
            def transpose_to(dst_name, src_tiles):
                dst = bpool.tile([128, L], f32, name=dst_name, tag=dst_name)
                for ti, (qs, qn) in enumerate(PT):
                    ps = ps_g.tile([128, 128], f32, name=f"ps_{dst_name}{ti}", tag="g")
                    nc.tensor.transpose(ps[:, 0:qn], src_tiles[ti][0:qn, :], ident[0:qn, 0:qn])
                    nc.scalar.copy(out=dst[:, qs:qs + qn], in_=ps[:, 0:qn])
                return dst
            qT = transpose_to("qT", q_sb)
            kT = transpose_to("kT", k_sb)
            pkT = transpose_to("pkT", pk_sb)

            # ---- projections ----
            # QsT[hd, l] = (Wq @ queries.T + bq) * 1/8
            psq = ps_g.tile([128, L], f32, name="psq", tag="g")
            nc.tensor.matmul(psq, wqt, qT, start=True, stop=True)
            qsT = bpool.tile([128, L], f32, name="qsT", tag="qsT")
            nc.vector.tensor_scalar(out=qsT, in0=psq, scalar1=bqc, scalar2=SCALE,
                                    op0=OP.add, op1=OP.mult)
            # KpT[hd, l] = Wk @ keys.T + bk + abs_pos_K.T
            psk = ps_g.tile([128, L], f32, name="psk", tag="g")
            nc.tensor.matmul(psk, wkt, kT, start=True, stop=True)
            kpT = bpool.tile([128, L], f32, name="kpT", tag="kpT")
            nc.vector.scalar_tensor_tensor(out=kpT, in0=psk, scalar=bkc, in1=pkT,
                                           op0=OP.add, op1=OP.add)
            # Qs[l, hd] per q-tile (scaled, for the K-side relation multiply)
            qs_tiles = []
            qsb_tiles = []
            for ti, (qs, qn) in enumerate(PT):
                psq2 = ps_g.tile([128, H], f32, name=f"psq2_{ti}", tag="g")
                nc.tensor.matmul(psq2[0:qn, :], qT[:, qs:qs + qn], wqt, start=True, stop=True)
                qst = bpool.tile([128, H], f32, name=f"qs{ti}", tag=f"qs{ti}")
                nc.vector.scalar_tensor_tensor(out=qst[0:qn, :], in0=psq2[0:qn, :],
                                               scalar=SCALE, in1=bq8r[0:qn, :],
                                               op0=OP.mult, op1=OP.add)
                qsb = bpool.tile([128, H], bf16, name=f"qsb{ti}", tag=f"qsb{ti}")
                nc.vector.tensor_copy(out=qsb[0:qn, :], in_=qst[0:qn, :])
                qs_tiles.append(qst)
                qsb_tiles.append(qsb)
            # Vp[l, hd] per k-tile: keys @ Wv.T + bv + abs_pos_V
            vp_tiles = []
            for ti, (ks, kn) in enumerate(PT):
                psv = ps_g.tile([128, H], f32, name=f"psv{ti}", tag="g")
                nc.tensor.matmul(psv[0:kn, :], kT[:, ks:ks + kn], wvt, start=True, stop=True)
                pvb = bpool.tile([128, H], f32, name=f"pvb{ti}", tag=f"pvb{ti}")
                nc.vector.tensor_add(out=pvb[0:kn, :], in0=pv_sb[ti][0:kn, :], in1=bvr[0:kn, :])
                vp = bpool.tile([128, H], f32, name=f"vp{ti}", tag=f"vp{ti}")
                nc.vector.tensor_add(out=vp[0:kn, :], in0=psv[0:kn, :], in1=pvb[0:kn, :])
                vp_tiles.append(vp)

            # ---- masks per q-tile ----
            notm_t, negm_t = [], []
            for ti, (qs, qn) in enumerate(PT):
                tmu = small.tile([128, 1], u8, name=f"tmu{ti}", tag="tmu")
                nc.gpsimd.dma_start(out=tmu[0:qn, :], in_=ap_of(d_tm, b * L + qs, [[1, qn], [0, 1]]))
                tmf = small.tile([128, 1], f32, name=f"tmf{ti}", tag="tmf")
                nc.vector.tensor_copy(out=tmf[0:qn, :], in_=tmu[0:qn, :])
                omt = small.tile([128, 1], f32, name=f"omt{ti}", tag="omt")
                nc.vector.tensor_scalar(out=omt[0:qn, :], in0=tmf[0:qn, :],
                                        scalar1=-1.0, scalar2=1.0, op0=OP.mult, op1=OP.add)
                notm = bpool.tile([128, L], f32, name=f"notm{ti}", tag=f"notm{ti}")
                nc.vector.tensor_scalar_mul(out=notm[0:qn, :], in0=omas[ti][0:qn, :],
                                            scalar1=omt[0:qn, :])
                negm = bpool.tile([128, L], f32, name=f"negm{ti}", tag=f"negm{ti}")
                nc.vector.tensor_scalar(out=negm[0:qn, :], in0=notm[0:qn, :],
                                        scalar1=-NEGC, scalar2=NEGC, op0=OP.mult, op1=OP.add)
                notm_t.append(notm); negm_t.append(negm)

            STATE.append(dict(qsT=qsT, kpT=kpT, qs_tiles=qs_tiles, qsb_tiles=qsb_tiles,
                              vp_tiles=vp_tiles, notm_t=notm_t, negm_t=negm_t))

        for b in range(B_SH):
            st = STATE[b]
            qsT = st["qsT"]; kpT = st["kpT"]; qs_tiles = st["qs_tiles"]
            qsb_tiles = st["qsb_tiles"]; vp_tiles = st["vp_tiles"]
            notm_t = st["notm_t"]; negm_t = st["negm_t"]
            # ---- interleaved per q-tile: K-side relation -> scores/softmax ->
            # A transposes -> A@V' -> V-side relation for that tile's q's.
            # This lets qtile-1 K streaming overlap qtile-0 V-side work.
            at_tiles = {}
            for h in range(NH):
                for kt in range(2):
                    at = bpool.tile([128, L], f32, name=f"at{h}{kt}", tag=f"at{h}{kt}")
                    at_tiles[(h, kt)] = at
            ei_tiles = []
            for kt in range(2):
                ei = bpool.tile([128, 2 * L], bf16, name=f"ei{kt}", tag=f"ei{kt}")
                ei_tiles.append(ei)
            relf1 = [bpool.tile([128, H], f32, name=f"relf1_{ti}", tag=f"relf1_{ti}")
                     for ti in range(2)]
            relf2 = [bpool.tile([128, H], f32, name=f"relf2_{ti}", tag=f"relf2_{ti}")
                     for ti in range(2)]
            avq_tiles = []

            qeng = [nc.sync, nc.gpsimd, nc.scalar, nc.gpsimd]
            for ti, (qs, qn) in enumerate(PT):
                # -- K-side relation for this q tile --
                srel = bpool.tile([128, 2 * L], f32, name=f"srel{ti}", tag=f"srel{ti}")
                for c in range(NKC):
                    tmk = kpool.tile([128, KC, H], bf16, name=f"tmk_{ti}_{c}", tag="tmk")
                    nc.gpsimd.dma_start(out=tmk[0:qn, :, :],
                                        in_=d_tmK[b, qs:qs + qn, c * KC:(c + 1) * KC, :])
                    dmk = kpool.tile([128, KC, H], bf16, name=f"dmk_{ti}_{c}", tag="dmk")
                    nc.gpsimd.dma_start(out=dmk[0:qn, :, :],
                                        in_=d_dmK[b, qs:qs + qn, c * KC:(c + 1) * KC, :])
                    prod = kpool.tile([128, KC, H], bf16, name=f"prod_{ti}_{c}", tag="prod")
                    nc.vector.tensor_add(out=prod[0:qn], in0=tmk[0:qn], in1=dmk[0:qn])
                    nc.vector.tensor_mul(out=prod[0:qn], in0=prod[0:qn],
                                         in1=bc_mid(qsb_tiles[ti][0:qn, :], KC))
                    nc.vector.reduce_sum(
                        out=srel[0:qn, 2 * KC * c:2 * KC * (c + 1)].rearrange(
                            "p (k h) -> p k h", h=NH),
                        in_=prod[0:qn].rearrange("p k (h d) -> p k h d", h=NH),
                        axis=AX.X)
                    # keep-alive matmul chained to this chunk: keeps the PE's
                    # HAM activity window busy through DVE-only phases so the
                    # V-side bursts run at the warm 2.4 GHz clock
                    ka = ps_g.tile([32, 2], f32, name=f"ka{ti}_{c}", tag="g")
                    nc.tensor.matmul(ka, prod[0:32, 0, 0:32], prod[0:32, 0, 0:2],
                                     start=True, stop=True)

                # -- scores, softmax, A transposes for this q tile --
                for h in range(NH):
                    psS = ps_g.tile([128, L], f32, name=f"psS{ti}{h}", tag="g")
                    nc.tensor.matmul(psS[0:qn, :], qsT[64 * h:64 * h + 64, qs:qs + qn],
                                     kpT[64 * h:64 * h + 64, :], start=True, stop=True)
                    s1 = small.tile([128, L], f32, name=f"s1_{ti}{h}", tag="s1")
                    srel_h = srel[0:qn, :].rearrange("p (k h) -> p k h", h=NH)[:, :, h]
                    nc.vector.tensor_add(out=s1[0:qn, :], in0=psS[0:qn, :], in1=srel_h)
                    nc.vector.tensor_mul(out=s1[0:qn, :], in0=s1[0:qn, :], in1=notm_t[ti][0:qn, :])
                    nc.vector.tensor_add(out=s1[0:qn, :], in0=s1[0:qn, :], in1=negm_t[ti][0:qn, :])
                    rmax = small.tile([128, 1], f32, name=f"rmax{ti}{h}", tag="rmax")
                    nc.vector.reduce_max(rmax[0:qn, :], s1[0:qn, :], axis=AX.X)
                    nrmax = small.tile([128, 1], f32, name=f"nrmax{ti}{h}", tag="nrmax")
                    nc.vector.tensor_scalar_mul(out=nrmax[0:qn, :], in0=rmax[0:qn, :], scalar1=-1.0)
                    a_t = small.tile([128, L], f32, name=f"a{ti}{h}", tag="a")
                    nc.scalar.activation(out=a_t[0:qn, :], in_=s1[0:qn, :], func=AF.Exp,
                                         bias=nrmax[0:qn, :], scale=1.0)
                    rsum = small.tile([128, 1], f32, name=f"rsum{ti}{h}", tag="rsum")
                    nc.vector.reduce_sum(rsum[0:qn, :], a_t[0:qn, :], axis=AX.X)
                    rinv = small.tile([128, 1], f32, name=f"rinv{ti}{h}", tag="rinv")
                    nc.vector.reciprocal(rinv[0:qn, :], rsum[0:qn, :])
                    nc.vector.tensor_scalar_mul(out=a_t[0:qn, :], in0=a_t[0:qn, :],
                                                scalar1=rinv[0:qn, :])
                    for kt, (ks, kn) in enumerate(PT):
                        pst = ps_g.tile([128, 128], f32, name=f"pst{ti}{h}{kt}", tag="g")
                        nc.tensor.transpose(pst[0:kn, 0:qn], a_t[0:qn, ks:ks + kn],
                                            ident[0:qn, 0:qn])
                        nc.scalar.copy(out=at_tiles[(h, kt)][0:kn, qs:qs + qn],
                                       in_=pst[0:kn, 0:qn])
                        ei_dst = ei_tiles[kt][0:kn, :].rearrange(
                            "p (q h) -> p q h", h=NH)[:, qs:qs + qn, h]
                        nc.scalar.copy(out=ei_dst, in_=pst[0:kn, 0:qn])

                # -- A @ V' for this q tile --
                avq = ps_avq.tile([128, H], f32, name=f"avq{ti}", tag="avq")
                for h in range(NH):
                    for kt, (ks, kn) in enumerate(PT):
                        nc.tensor.matmul(avq[0:qn, 64 * h:64 * h + 64],
                                         at_tiles[(h, kt)][0:kn, qs:qs + qn],
                                         vp_tiles[kt][0:kn, 64 * h:64 * h + 64],
                                         start=(kt == 0), stop=(kt == 1))
                avq_tiles.append(avq)

                # -- V-side relation for this q tile's q's --
                for g16 in range((qn + 15) // 16):
                    q0 = qs + 16 * g16
                    qn16 = min(16, qs + qn - q0)
                    lo = q0 - qs
                    stg = vpool.tile([2, 16 * 2 * H], f32, name=f"stg{ti}_{g16}", tag="stg", bufs=2)
                    td16 = []
                    for kt, (ks, kn) in enumerate(PT):
                        td = vpool.tile([128, 16, 2, H], bf16, name=f"td{kt}_{q0}",
                                        tag=f"td{kt}", bufs=2)
                        for xi, dram in enumerate((d_tmV, d_dmV)):
                            nc.gpsimd.dma_start(
                                out=td[0:kn, 0:qn16, xi, :],
                                in_=dram[b, q0:q0 + qn16, ks:ks + kn, :].rearrange(
                                    "q k h -> k q h"))
                        td16.append(td)
                    for g4 in range((qn16 + 3) // 4):
                        q4 = q0 + 4 * g4
                        qn4 = min(4, qs + qn - q4)
                        rp = ps_rp.tile([2, QG * 2 * H], f32, name=f"rp{q4}", tag="rp")
                        for ql in range(qn4):
                            q = q4 + ql
                            rpo = rp[:, ql * 2 * H:(ql + 1) * 2 * H]
                            nc.tensor.matmul(rpo, ei_tiles[0][0:128, 2 * q:2 * q + 2],
                                             td16[0][:, q - q0, :, :],
                                             start=True, stop=False)
                            nc.tensor.matmul(rpo, ei_tiles[1][0:72, 2 * q:2 * q + 2],
                                             td16[1][0:72, q - q0, :, :],
                                             start=False, stop=True)
                        off = (q4 - q0) * 2 * H
                        nc.scalar.copy(out=stg[:, off:off + qn4 * 2 * H],
                                       in_=rp[:, 0:qn4 * 2 * H])
                    stg_v = stg.rearrange("p (q t c) -> p q t c", t=4, c=64)[:, 0:qn16]
                    nc.sync.dma_start(out=relf1[ti][lo:lo + qn16, 0:64], in_=stg_v[0:1, :, 0, :])
                    nc.sync.dma_start(out=relf1[ti][lo:lo + qn16, 64:128], in_=stg_v[1:2, :, 1, :])
                    nc.sync.dma_start(out=relf2[ti][lo:lo + qn16, 0:64], in_=stg_v[0:1, :, 2, :])
                    nc.sync.dma_start(out=relf2[ti][lo:lo + qn16, 64:128], in_=stg_v[1:2, :, 3, :])

            # ---- combine and store ----
            for ti, (qs, qn) in enumerate(PT):
                rsum2 = bpool.tile([128, H], f32, name=f"rsum2_{ti}", tag=f"rsum2_{ti}")
                nc.vector.scalar_tensor_tensor(out=rsum2[0:qn, :], in0=relf1[ti][0:qn, :],
                                               scalar=1.0, in1=relf2[ti][0:qn, :],
                                               op0=OP.bypass, op1=OP.add)
                outs = bpool.tile([128, H], f32, name=f"outs{ti}", tag=f"outs{ti}")
                nc.vector.tensor_add(out=outs[0:qn, :], in0=avq_tiles[ti][0:qn, :],
                                     in1=rsum2[0:qn, :])
                nc.sync.dma_start(out=d_out[b, qs:qs + qn, :], in_=outs[0:qn, :])

    nc.compile()
    return nc


def _get_nc():
    if "nc" not in _CACHE:
        _CACHE["nc"] = _build_nc()
    return _CACHE["nc"]


def _shard_inputs(inputs):
    """Build the 8 per-core input maps (batch-sharded, weights replicated)."""
    inp = {k: np.asarray(v) for k, v in inputs.items()}
    batch_keys = ("queries", "keys", "time_matrix_K", "time_matrix_V",
                  "dis_matrix_K", "dis_matrix_V", "abs_pos_K", "abs_pos_V")
    rep_keys = ("Wq", "bq", "Wk", "bk", "Wv", "bv")
    in_maps = []
    for c in range(NCORES):
        sl = slice(c * B_SH, (c + 1) * B_SH)
        m = {}
        for k in batch_keys:
            m[k] = np.ascontiguousarray(inp[k][sl], dtype=np.float32)
        for k in rep_keys:
            m[k] = np.ascontiguousarray(inp[k], dtype=np.float32)
        m["time_mask"] = np.ascontiguousarray(inp["time_mask"][sl]).astype(np.uint8)
        m["attn_mask"] = np.ascontiguousarray(inp["attn_mask"]).astype(np.uint8)
        in_maps.append(m)
    return in_maps


def run(inputs, trace=False, tmpdir=None):
    """Run on the 8 NeuronCores; returns (full_output, BassKernelResults)."""
    from concourse.bass_utils import run_bass_kernel_spmd
    nc = _get_nc()
    in_maps = _shard_inputs(inputs)
    res = run_bass_kernel_spmd(nc, in_maps, core_ids=list(range(NCORES)),
                               trace=trace, tmpdir=tmpdir)
    out = np.concatenate([r["out"] for r in res.results], axis=0)
    return out.astype(np.float32), res


def kernel(**inputs) -> np.ndarray:
    out, _ = run(inputs, trace=False)
    return out


# revision 15
# speedup vs baseline: 1.0078x; 1.0078x over previous
"""Trainium2 Bass kernel for the AGRAN anchor attention module.

Reference math (per batch b, heads h=2, d=64, L=200, H=128):
  Q = (queries @ Wq.T + bq) * 1/sqrt(d)          [b, l, h*d]
  K' = keys @ Wk.T + bk + abs_pos_K              [b, l, h*d]
  V' = keys @ Wv.T + bv + abs_pos_V              [b, l, h*d]
  S[h,q,k] = Q_h[q]·K'_h[k] + sum_d (tmK+dmK)[q,k,h,d]·Q_h[q,d]
  S masked by (time_mask row | causal), softmax over k -> A
  out[h,q] = sum_k A[h,q,k] * (V'_h[k] + (tmV+dmV)[q,k,h,:])

Sharding: data-parallel over batch B=16 across 8 cores (2 per core),
weights/masks replicated.  No collectives; the host concatenates shards.

Per-core dataflow:
  * K-side relation tensors stream as [q-partition, (k,hd)-free] chunks
    through the vector engine (add, broadcast-mult by scaled Q, segmented
    reduce over d).
  * V-side relation tensors load as per-(b,q) [k-partition, tm|dm] tiles;
    PE contracts over k with 2-column A slices as stationary weights
    (float32r, single pass).  The [2, 256] PSUM strips are scattered by
    GPSIMD DMAs into [q, hd] tiles.
  * DMA issue is spread over the Sync/Act HWDGE rings and 8 SWDGE queues.
"""

import numpy as np

B, L, H, NH, D = 16, 200, 128, 2, 64
NCORES = 8
B_SH = B // NCORES  # 2 batches per core
NEGC = -30000.0     # masked-score fill (any row-constant that underflows exp)
SCALE = 0.125       # 1/sqrt(D)
PT = [(0, 128), (128, 72)]  # partition tiling of the 200-long seq dim
KC = 25             # k-chunk for the K-side relation streaming
NKC = L // KC       # 8 chunks
QG = 4              # q's per PSUM strip tile on the V side

_CACHE = {}


def _build_nc():
    import concourse.bass as bass
    import concourse.bacc as bacc
    import concourse.tile as tile
    from concourse import mybir
    from concourse.masks import make_identity
    from contextlib import ExitStack

    f32 = mybir.dt.float32
    f32r = mybir.dt.float32r
    bf16 = mybir.dt.bfloat16
    u8 = mybir.dt.uint8
    AF = mybir.ActivationFunctionType
    OP = mybir.AluOpType
    AX = mybir.AxisListType

    nc = bacc.Bacc(num_swdge_queues=4)

    d_q = nc.declare_dram_parameter("queries", [B_SH, L, H], f32, isOutput=False)
    d_k = nc.declare_dram_parameter("keys", [B_SH, L, H], f32, isOutput=False)
    d_tmK = nc.declare_dram_parameter("time_matrix_K", [B_SH, L, L, H], f32, isOutput=False)
    d_tmV = nc.declare_dram_parameter("time_matrix_V", [B_SH, L, L, H], f32, isOutput=False)
    d_dmK = nc.declare_dram_parameter("dis_matrix_K", [B_SH, L, L, H], f32, isOutput=False)
    d_dmV = nc.declare_dram_parameter("dis_matrix_V", [B_SH, L, L, H], f32, isOutput=False)
    d_pK = nc.declare_dram_parameter("abs_pos_K", [B_SH, L, H], f32, isOutput=False)
    d_pV = nc.declare_dram_parameter("abs_pos_V", [B_SH, L, H], f32, isOutput=False)
    d_wq = nc.declare_dram_parameter("Wq", [H, H], f32, isOutput=False)
    d_bq = nc.declare_dram_parameter("bq", [H], f32, isOutput=False)
    d_wk = nc.declare_dram_parameter("Wk", [H, H], f32, isOutput=False)
    d_bk = nc.declare_dram_parameter("bk", [H], f32, isOutput=False)
    d_wv = nc.declare_dram_parameter("Wv", [H, H], f32, isOutput=False)
    d_bv = nc.declare_dram_parameter("bv", [H], f32, isOutput=False)
    d_tm = nc.declare_dram_parameter("time_mask", [B_SH, L], u8, isOutput=False)
    d_am = nc.declare_dram_parameter("attn_mask", [L, L], u8, isOutput=False)
    d_out = nc.declare_dram_parameter("out", [B_SH, L, H], f32, isOutput=True)

    def ap_of(t, offset, pattern):
        return bass.AP(tensor=t.tensor if isinstance(t, bass.AP) else t, offset=offset, ap=pattern)

    def bc_mid(ap2d, count):
        """[P, F] -> [P, count(bcast), F] by inserting a step-0 free dim."""
        return bass.AP(tensor=ap2d.tensor, offset=ap2d.offset,
                       ap=[ap2d.ap[0], [0, count], ap2d.ap[1]])

    with tile.TileContext(nc) as tc, ExitStack() as ctx:
        consts = ctx.enter_context(tc.tile_pool(name="consts", bufs=1))
        bpool = ctx.enter_context(tc.tile_pool(name="bpool", bufs=2))
        kpool = ctx.enter_context(tc.tile_pool(name="kpool", bufs=2))
        vpool = ctx.enter_context(tc.tile_pool(name="vpool", bufs=6))
        small = ctx.enter_context(tc.tile_pool(name="small", bufs=3))
        ps_g = ctx.enter_context(tc.tile_pool(name="ps_g", bufs=2, space="PSUM"))
        ps_avq = ctx.enter_context(tc.tile_pool(name="ps_avq", bufs=2, space="PSUM"))
        ps_rp = ctx.enter_context(tc.tile_pool(name="ps_rp", bufs=2, space="PSUM"))

        # ---- constants ----
        ident = consts.tile([128, 128], f32, name="ident", tag="ident")
        make_identity(nc, ident)

        wt_tiles = []
        for nm, dram in (("wq", d_wq), ("wk", d_wk), ("wv", d_wv)):
            wnat = consts.tile([128, 128], f32, name=f"{nm}nat", tag=f"{nm}nat")
            nc.gpsimd.dma_start(out=wnat, in_=dram[:, :])
            wps = ps_g.tile([128, 128], f32, name=f"{nm}ps", tag="g")
            nc.tensor.transpose(wps, wnat, ident)
            wt = consts.tile([128, 128], f32, name=f"{nm}t", tag=f"{nm}t")
            nc.scalar.copy(out=wt, in_=wps)
            wt_tiles.append(wt)
        wqt, wkt, wvt = wt_tiles

        # bq broadcast to all partitions, pre-scaled by 1/8 (free-dim layout)
        bq8r = consts.tile([128, 128], f32, name="bq8r", tag="bq8r")
        nc.gpsimd.dma_start(out=bq8r, in_=ap_of(d_bq, 0, [[0, 128], [1, 128]]))
        nc.scalar.mul(out=bq8r, in_=bq8r, mul=SCALE)
        # bv broadcast (free-dim layout)
        bvr = consts.tile([128, 128], f32, name="bvr", tag="bvr")
        nc.gpsimd.dma_start(out=bvr, in_=ap_of(d_bv, 0, [[0, 128], [1, 128]]))
        # per-partition column biases ([H,1])
        bqc = consts.tile([128, 1], f32, name="bqc", tag="bqc")
        nc.gpsimd.dma_start(out=bqc, in_=ap_of(d_bq, 0, [[1, 128], [0, 1]]))
        bkc = consts.tile([128, 1], f32, name="bkc", tag="bkc")
        nc.gpsimd.dma_start(out=bkc, in_=ap_of(d_bk, 0, [[1, 128], [0, 1]]))

        # (1 - attn_mask) per q-tile (shared across b)
        omas = []
        for ti, (qs, qn) in enumerate(PT):
            amu = consts.tile([128, L], u8, name=f"amu{ti}", tag=f"amu{ti}")
            nc.gpsimd.dma_start(out=amu[0:qn, :], in_=d_am[qs:qs + qn, :])
            oma = consts.tile([128, L], f32, name=f"oma{ti}", tag=f"oma{ti}")
            nc.vector.tensor_copy(out=oma[0:qn, :], in_=amu[0:qn, :])
            nc.vector.tensor_scalar(out=oma[0:qn, :], in0=oma[0:qn, :],
                                    scalar1=-1.0, scalar2=1.0,
                                    op0=OP.mult, op1=OP.add)
            omas.append(oma)

        STATE = []
        for b in range(B_SH):
            # ---- load activations [l, hd] per seq tile ----
            q_sb, k_sb, pk_sb, pv_sb = [], [], [], []
            for ti, (qs, qn) in enumerate(PT):
                tq = bpool.tile([128, H], f32, name=f"tq{ti}", tag=f"tq{ti}")
                nc.gpsimd.dma_start(out=tq[0:qn, :], in_=d_q[b, qs:qs + qn, :])
                tk = bpool.tile([128, H], f32, name=f"tk{ti}", tag=f"tk{ti}")
                nc.gpsimd.dma_start(out=tk[0:qn, :], in_=d_k[b, qs:qs + qn, :])
                tpk = bpool.tile([128, H], f32, name=f"tpk{ti}", tag=f"tpk{ti}")
                nc.gpsimd.dma_start(out=tpk[0:qn, :], in_=d_pK[b, qs:qs + qn, :])
                tpv = bpool.tile([128, H], f32, name=f"tpv{ti}", tag=f"tpv{ti}")
                nc.gpsimd.dma_start(out=tpv[0:qn, :], in_=d_pV[b, qs:qs + qn, :])
                q_sb.append(tq); k_sb.append(tk); pk_sb.append(tpk); pv_sb.append(tpv)

            # ---- transpose activations to [hd, l] ----
            def transpose_to(dst_name, src_tiles):
                dst = bpool.tile([128, L], f32, name=dst_name, tag=dst_name)
                for ti, (qs, qn) in enumerate(PT):
                    ps = ps_g.tile([128, 128], f32, name=f"ps_{dst_name}{ti}", tag="g")
                    nc.tensor.transpose(ps[:, 0:qn], src_tiles[ti][0:qn, :], ident[0:qn, 0:qn])
                    nc.scalar.copy(out=dst[:, qs:qs + qn], in_=ps[:, 0:qn])
                return dst
            qT = transpose_to("qT", q_sb)
            kT = transpose_to("kT", k_sb)
            pkT = transpose_to("pkT", pk_sb)

            # ---- projections ----
            # QsT[hd, l] = (Wq @ queries.T + bq) * 1/8
            psq = ps_g.tile([128, L], f32, name="psq", tag="g")
            nc.tensor.matmul(psq, wqt, qT, start=True, stop=True)
            qsT = bpool.tile([128, L], f32, name="qsT", tag="qsT")
            nc.vector.tensor_scalar(out=qsT, in0=psq, scalar1=bqc, scalar2=SCALE,
                                    op0=OP.add, op1=OP.mult)
            # KpT[hd, l] = Wk @ keys.T + bk + abs_pos_K.T
            psk = ps_g.tile([128, L], f32, name="psk", tag="g")
            nc.tensor.matmul(psk, wkt, kT, start=True, stop=True)
            kpT = bpool.tile([128, L], f32, name="kpT", tag="kpT")
            nc.vector.scalar_tensor_tensor(out=kpT, in0=psk, scalar=bkc, in1=pkT,
                                           op0=OP.add, op1=OP.add)
            # Qs[l, hd] per q-tile (scaled, for the K-side relation multiply)
            qs_tiles = []
            qsb_tiles = []
            for ti, (qs, qn) in enumerate(PT):
                psq2 = ps_g.tile([128, H], f32, name=f"psq2_{ti}", tag="g")
                nc.tensor.matmul(psq2[0:qn, :], qT[:, qs:qs + qn], wqt, start=True, stop=True)
                qst = bpool.tile([128, H], f32, name=f"qs{ti}", tag=f"qs{ti}")
                nc.vector.scalar_tensor_tensor(out=qst[0:qn, :], in0=psq2[0:qn, :],
                                               scalar=SCALE, in1=bq8r[0:qn, :],
                                               op0=OP.mult, op1=OP.add)
                qsb = bpool.tile([128, H], bf16, name=f"qsb{ti}", tag=f"qsb{ti}")
                nc.vector.tensor_copy(out=qsb[0:qn, :], in_=qst[0:qn, :])
                qs_tiles.append(qst)
                qsb_tiles.append(qsb)
            # Vp[l, hd] per k-tile: keys @ Wv.T + bv + abs_pos_V
            vp_tiles = []
            for ti, (ks, kn) in enumerate(PT):
                psv = ps_g.tile([128, H], f32, name=f"psv{ti}", tag="g")
                nc.tensor.matmul(psv[0:kn, :], kT[:, ks:ks + kn], wvt, start=True, stop=True)
                pvb = bpool.tile([128, H], f32, name=f"pvb{ti}", tag=f"pvb{ti}")
                nc.vector.tensor_add(out=pvb[0:kn, :], in0=pv_sb[ti][0:kn, :], in1=bvr[0:kn, :])
                vp = bpool.tile([128, H], f32, name=f"vp{ti}", tag=f"vp{ti}")
                nc.vector.tensor_add(out=vp[0:kn, :], in0=psv[0:kn, :], in1=pvb[0:kn, :])
                vp_tiles.append(vp)

            # ---- masks per q-tile ----
            notm_t, negm_t = [], []
            for ti, (qs, qn) in enumerate(PT):
                tmu = small.tile([128, 1], u8, name=f"tmu{ti}", tag="tmu")
                nc.gpsimd.dma_start(out=tmu[0:qn, :], in_=ap_of(d_tm, b * L + qs, [[1, qn], [0, 1]]))
                tmf = small.tile([128, 1], f32, name=f"tmf{ti}", tag="tmf")
                nc.vector.tensor_copy(out=tmf[0:qn, :], in_=tmu[0:qn, :])
                omt = small.tile([128, 1], f32, name=f"omt{ti}", tag="omt")
                nc.vector.tensor_scalar(out=omt[0:qn, :], in0=tmf[0:qn, :],
                                        scalar1=-1.0, scalar2=1.0, op0=OP.mult, op1=OP.add)
                notm = bpool.tile([128, L], f32, name=f"notm{ti}", tag=f"notm{ti}")
                nc.vector.tensor_scalar_mul(out=notm[0:qn, :], in0=omas[ti][0:qn, :],
                                            scalar1=omt[0:qn, :])
                negm = bpool.tile([128, L], f32, name=f"negm{ti}", tag=f"negm{ti}")
                nc.vector.tensor_scalar(out=negm[0:qn, :], in0=notm[0:qn, :],
                                        scalar1=-NEGC, scalar2=NEGC, op0=OP.mult, op1=OP.add)
                notm_t.append(notm); negm_t.append(negm)

            STATE.append(dict(qsT=qsT, kpT=kpT, qs_tiles=qs_tiles, qsb_tiles=qsb_tiles,
                              vp_tiles=vp_tiles, notm_t=notm_t, negm_t=negm_t))

        for b in range(B_SH):
            st = STATE[b]
            qsT = st["qsT"]; kpT = st["kpT"]; qs_tiles = st["qs_tiles"]
            qsb_tiles = st["qsb_tiles"]; vp_tiles = st["vp_tiles"]
            notm_t = st["notm_t"]; negm_t = st["negm_t"]
            # ---- interleaved per q-tile: K-side relation -> scores/softmax ->
            # A transposes -> A@V' -> V-side relation for that tile's q's.
            # This lets qtile-1 K streaming overlap qtile-0 V-side work.
            at_tiles = {}
            for h in range(NH):
                for kt in range(2):
                    at = bpool.tile([128, L], f32, name=f"at{h}{kt}", tag=f"at{h}{kt}")
                    at_tiles[(h, kt)] = at
            ei_tiles = []
            for kt in range(2):
                ei = bpool.tile([128, 2 * L], bf16, name=f"ei{kt}", tag=f"ei{kt}")
                ei_tiles.append(ei)
            relf1 = [bpool.tile([128, H], f32, name=f"relf1_{ti}", tag=f"relf1_{ti}")
                     for ti in range(2)]
            relf2 = [bpool.tile([128, H], f32, name=f"relf2_{ti}", tag=f"relf2_{ti}")
                     for ti in range(2)]
            avq_tiles = []

            qeng = [nc.sync, nc.gpsimd, nc.scalar, nc.gpsimd]
            for ti, (qs, qn) in enumerate(PT):
                # -- K-side relation for this q tile --
                srel = bpool.tile([128, 2 * L], f32, name=f"srel{ti}", tag=f"srel{ti}")
                for c in range(NKC):
                    tmk = kpool.tile([128, KC, H], bf16, name=f"tmk_{ti}_{c}", tag="tmk", bufs=4)
                    nc.gpsimd.dma_start(out=tmk[0:qn, :, :],
                                        in_=d_tmK[b, qs:qs + qn, c * KC:(c + 1) * KC, :])
                    dmk = kpool.tile([128, KC, H], bf16, name=f"dmk_{ti}_{c}", tag="dmk", bufs=4)
                    nc.gpsimd.dma_start(out=dmk[0:qn, :, :],
                                        in_=d_dmK[b, qs:qs + qn, c * KC:(c + 1) * KC, :])
                    prod = kpool.tile([128, KC, H], bf16, name=f"prod_{ti}_{c}", tag="prod")
                    nc.vector.tensor_add(out=prod[0:qn], in0=tmk[0:qn], in1=dmk[0:qn])
                    nc.vector.tensor_mul(out=prod[0:qn], in0=prod[0:qn],
                                         in1=bc_mid(qsb_tiles[ti][0:qn, :], KC))
                    nc.vector.reduce_sum(
                        out=srel[0:qn, 2 * KC * c:2 * KC * (c + 1)].rearrange(
                            "p (k h) -> p k h", h=NH),
                        in_=prod[0:qn].rearrange("p k (h d) -> p k h d", h=NH),
                        axis=AX.X)

                # -- scores, softmax, A transposes for this q tile --
                for h in range(NH):
                    psS = ps_g.tile([128, L], f32, name=f"psS{ti}{h}", tag="g")
                    nc.tensor.matmul(psS[0:qn, :], qsT[64 * h:64 * h + 64, qs:qs + qn],
                                     kpT[64 * h:64 * h + 64, :], start=True, stop=True)
                    s1 = small.tile([128, L], f32, name=f"s1_{ti}{h}", tag="s1")
                    srel_h = srel[0:qn, :].rearrange("p (k h) -> p k h", h=NH)[:, :, h]
                    nc.vector.tensor_add(out=s1[0:qn, :], in0=psS[0:qn, :], in1=srel_h)
                    nc.vector.tensor_mul(out=s1[0:qn, :], in0=s1[0:qn, :], in1=notm_t[ti][0:qn, :])
                    nc.vector.tensor_add(out=s1[0:qn, :], in0=s1[0:qn, :], in1=negm_t[ti][0:qn, :])
                    rmax = small.tile([128, 1], f32, name=f"rmax{ti}{h}", tag="rmax")
                    nc.vector.reduce_max(rmax[0:qn, :], s1[0:qn, :], axis=AX.X)
                    nrmax = small.tile([128, 1], f32, name=f"nrmax{ti}{h}", tag="nrmax")
                    nc.vector.tensor_scalar_mul(out=nrmax[0:qn, :], in0=rmax[0:qn, :], scalar1=-1.0)
                    a_t = small.tile([128, L], f32, name=f"a{ti}{h}", tag="a")
                    nc.scalar.activation(out=a_t[0:qn, :], in_=s1[0:qn, :], func=AF.Exp,
                                         bias=nrmax[0:qn, :], scale=1.0)
                    rsum = small.tile([128, 1], f32, name=f"rsum{ti}{h}", tag="rsum")
                    nc.vector.reduce_sum(rsum[0:qn, :], a_t[0:qn, :], axis=AX.X)
                    rinv = small.tile([128, 1], f32, name=f"rinv{ti}{h}", tag="rinv")
                    nc.vector.reciprocal(rinv[0:qn, :], rsum[0:qn, :])
                    nc.vector.tensor_scalar_mul(out=a_t[0:qn, :], in0=a_t[0:qn, :],
                                                scalar1=rinv[0:qn, :])
                    for kt, (ks, kn) in enumerate(PT):
                        pst = ps_g.tile([128, 128], f32, name=f"pst{ti}{h}{kt}", tag="g")
                        nc.tensor.transpose(pst[0:kn, 0:qn], a_t[0:qn, ks:ks + kn],
                                            ident[0:qn, 0:qn])
                        nc.scalar.copy(out=at_tiles[(h, kt)][0:kn, qs:qs + qn],
                                       in_=pst[0:kn, 0:qn])
                        ei_dst = ei_tiles[kt][0:kn, :].rearrange(
                            "p (q h) -> p q h", h=NH)[:, qs:qs + qn, h]
                        nc.scalar.copy(out=ei_dst, in_=pst[0:kn, 0:qn])

                # -- A @ V' for this q tile --
                avq = ps_avq.tile([128, H], f32, name=f"avq{ti}", tag="avq")
                for h in range(NH):
                    for kt, (ks, kn) in enumerate(PT):
                        nc.tensor.matmul(avq[0:qn, 64 * h:64 * h + 64],
                                         at_tiles[(h, kt)][0:kn, qs:qs + qn],
                                         vp_tiles[kt][0:kn, 64 * h:64 * h + 64],
                                         start=(kt == 0), stop=(kt == 1))
                avq_tiles.append(avq)

                # -- V-side relation for this q tile's q's --
                for g16 in range((qn + 15) // 16):
                    q0 = qs + 16 * g16
                    qn16 = min(16, qs + qn - q0)
                    lo = q0 - qs
                    stg = vpool.tile([2, 16 * 2 * H], f32, name=f"stg{ti}_{g16}", tag="stg", bufs=2)
                    td16 = []
                    for kt, (ks, kn) in enumerate(PT):
                        td = vpool.tile([128, 16, 2, H], bf16, name=f"td{kt}_{q0}",
                                        tag=f"td{kt}", bufs=2)
                        for xi, dram in enumerate((d_tmV, d_dmV)):
                            nc.gpsimd.dma_start(
                                out=td[0:kn, 0:qn16, xi, :],
                                in_=dram[b, q0:q0 + qn16, ks:ks + kn, :].rearrange(
                                    "q k h -> k q h"))
                        td16.append(td)
                    for g4 in range((qn16 + 3) // 4):
                        q4 = q0 + 4 * g4
                        qn4 = min(4, qs + qn - q4)
                        rp = ps_rp.tile([2, QG * 2 * H], f32, name=f"rp{q4}", tag="rp")
                        for ql in range(qn4):
                            q = q4 + ql
                            rpo = rp[:, ql * 2 * H:(ql + 1) * 2 * H]
                            nc.tensor.matmul(rpo, ei_tiles[0][0:128, 2 * q:2 * q + 2],
                                             td16[0][:, q - q0, :, :],
                                             start=True, stop=False)
                            nc.tensor.matmul(rpo, ei_tiles[1][0:72, 2 * q:2 * q + 2],
                                             td16[1][0:72, q - q0, :, :],
                                             start=False, stop=True)
                        off = (q4 - q0) * 2 * H
                        nc.scalar.copy(out=stg[:, off:off + qn4 * 2 * H],
                                       in_=rp[:, 0:qn4 * 2 * H])
                    stg_v = stg.rearrange("p (q t c) -> p q t c", t=4, c=64)[:, 0:qn16]
                    nc.sync.dma_start(out=relf1[ti][lo:lo + qn16, 0:64], in_=stg_v[0:1, :, 0, :])
                    nc.sync.dma_start(out=relf1[ti][lo:lo + qn16, 64:128], in_=stg_v[1:2, :, 1, :])
                    nc.sync.dma_start(out=relf2[ti][lo:lo + qn16, 0:64], in_=stg_v[0:1, :, 2, :])
                    nc.sync.dma_start(out=relf2[ti][lo:lo + qn16, 64:128], in_=stg_v[1:2, :, 3, :])

            # ---- combine and store ----
            for ti, (qs, qn) in enumerate(PT):
                rsum2 = bpool.tile([128, H], f32, name=f"rsum2_{ti}", tag=f"rsum2_{ti}")
                nc.vector.scalar_tensor_tensor(out=rsum2[0:qn, :], in0=relf1[ti][0:qn, :],
                                               scalar=1.0, in1=relf2[ti][0:qn, :],
                                               op0=OP.bypass, op1=OP.add)
                outs = bpool.tile([128, H], f32, name=f"outs{ti}", tag=f"outs{ti}")
                nc.vector.tensor_add(out=outs[0:qn, :], in0=avq_tiles[ti][0:qn, :],
                                     in1=rsum2[0:qn, :])
                nc.sync.dma_start(out=d_out[b, qs:qs + qn, :], in_=outs[0:qn, :])

    nc.compile()
    return nc


def _get_nc():
    if "nc" not in _CACHE:
        _CACHE["nc"] = _build_nc()
    return _CACHE["nc"]


def _shard_inputs(inputs):
    """Build the 8 per-core input maps (batch-sharded, weights replicated)."""
    inp = {k: np.asarray(v) for k, v in inputs.items()}
    batch_keys = ("queries", "keys", "time_matrix_K", "time_matrix_V",
                  "dis_matrix_K", "dis_matrix_V", "abs_pos_K", "abs_pos_V")
    rep_keys = ("Wq", "bq", "Wk", "bk", "Wv", "bv")
    in_maps = []
    for c in range(NCORES):
        sl = slice(c * B_SH, (c + 1) * B_SH)
        m = {}
        for k in batch_keys:
            m[k] = np.ascontiguousarray(inp[k][sl], dtype=np.float32)
        for k in rep_keys:
            m[k] = np.ascontiguousarray(inp[k], dtype=np.float32)
        m["time_mask"] = np.ascontiguousarray(inp["time_mask"][sl]).astype(np.uint8)
        m["attn_mask"] = np.ascontiguousarray(inp["attn_mask"]).astype(np.uint8)
        in_maps.append(m)
    return in_maps


def run(inputs, trace=False, tmpdir=None):
    """Run on the 8 NeuronCores; returns (full_output, BassKernelResults)."""
    from concourse.bass_utils import run_bass_kernel_spmd
    nc = _get_nc()
    in_maps = _shard_inputs(inputs)
    res = run_bass_kernel_spmd(nc, in_maps, core_ids=list(range(NCORES)),
                               trace=trace, tmpdir=tmpdir)
    out = np.concatenate([r["out"] for r in res.results], axis=0)
    return out.astype(np.float32), res


def kernel(**inputs) -> np.ndarray:
    out, _ = run(inputs, trace=False)
    return out
